# revision 29
# baseline (speedup 1.0000x reference)
"""Trainium2 Bass kernel for the supervoxel erode/edge loss module.

The reference pads a [B,X,Y] grid (offset 4*sx rows / 4*sy cols), tiles it
into 8x8 patches, zeroes each patch's last row/col of the mask channel,
erodes along both patch axes and sums eroded*edge.  The erode
`a*b + (1-a)*a + (1-b)*a` equals `2a - a^2 = 1 - (1-a)^2` with
a = m(i)*m(i+1), so the whole module collapses to a global elementwise
expression on the unpadded grid (validated to f64 exactness):

    mt(x,y) = mask[b,x,y,idx] * [(x+4sx)%8 != 7] * [(y+4sy)%8 != 7]
    ax = mt(x,y)*mt(x+1,y); ay = mt(x,y)*mt(x,y+1)   (zero past image edge)
    total = sum (1-(1-ax)^2) * (1-(1-ay)^2) * edge
    out = loss_old + total / (B * ((X+8)//8) * ((Y+8)//8))

Host-side packing (layout/precision only -- all arithmetic on device):
  * only mask channel `idx` matters (4x traffic cut), and mask rows with
    x%8 == 7-4sx / cols with y%8 == 7-4sy are provably dead, as are edge
    rows x%8 in {6-4sx,7-4sx} and cols y%8 in {6-4sy,7-4sy}.
  * f32 -> f16: inputs are U[0,1) and the loss is a mean of ~10M terms;
    measured end-to-end rel err ~5e-6 (gate is 2e-2).
  mask -> [Bc, 897, 897] f16: 7 live row/col classes per group of 8, plus
    one zero pad row and col (exact zero shift-ins at image edges).
  edge -> [Bc, 896, 768] f16 (for sy==0): 7 row slots per group aligned
    with the mask slots -- dead-term slots are zero rows -- by 6 live col
    classes.  For sy==1 cols are 7-wide with the dead column zeroed.

Device layout: partition p = row-group p (128 groups per image).  Each
partition holds 8 consecutive packed mask rows (7 + 1 overlap row = the
next group's first row), so a whole image's mask is 128 contiguous ~11KiB
DMA descriptors, split into per-live-range chunks (rows 0:3 / 3:8 for
sx=1, no rows read twice) so compute starts as soon as the first chunk
lands.  Products are emitted only for live term slots; the dead slot's
ax0/ay0 slices are zeroed once during the DMA ramp so the downstream
full-slot ops see exact zeros.  ALL input DMAs ride the sync
HWDGE ring (FIFO per engine) in exact consumption order -- mask chunks
for every image first, then edges (only needed by the join) -- so the
mask stream is never starved by edge traffic.  Both neighbor shifts are
free-dim views; no PE/transpose work at all:

    v = tile[:, 0:7, g, 0:6]   vx = tile[:, 1:8, g, 0:6]   vy = v at col+1

Engine assignment (measured on HW): GpSimd is avoided entirely -- any
GpSimd op locks the shared SBUF port pair that DVE needs for its second
tensor operand and fully blocks it.  scalar_tensor_tensor runs at 1x
(no 2x uop) so it appears exactly once, as the final op, where its free
in-op accum_out beats a separate pass; everything else is 2x/4x DVE or
ACT.  Per image, emitted y-chain first (the join consumes sqy first):

    DVE : ay0 = v*vy; ax0 = v*vx          (tensor_tensor, 2x f16)
    ACT : sqy = (1-ay0)^2; sqx = (1-ax0)^2  (Square activation)
    DVE : qy = sqy-1 (tensor_scalar 4x); t1 = qy*e (2x)
    last image : p2 = (sqx-1)*t1 [STT 1x, accum_out = the partial sum]
    earlier    : qx = sqx-1 (4x); p2 = qx*t1 (2x); sum via ACT Copy
                 accum_out (overlaps later images' DVE work)
    (sign flips cancel: (sqx-1)(sqy-1) = (1-sqx)(1-sqy))

A tiny reduce + PE ones-matmul then yields the scalar partial sum.

Sharding: data-parallel over batch, B/8 images per core on 8 cores; the
per-core partial sums combine on host (the mean needs no collective).

Progression on HW (exec_time, core 0): 354.8us (f32 tiled baseline) ->
61.1us (packed-f16 rewrite) -> 57.0us (hybrid join, no GpSimd) ->
53.4us (single-ring DMA ordering) -> 51.5us (y-first) -> 49.8us (SC=3)
-> 48.8us (dead-slot products skipped; chunk DMAs with no overlap row)
-> 48.7us (single mask DMA for later images; final STT sliced per range).
"""

import os
import sys

sys.path.insert(0, "/opt/trn_rl_repo")

import numpy as np

from concourse import bacc, bass, mybir, tile
from concourse.ap import AP
from concourse.bass_utils import run_bass_kernel_spmd

F32 = mybir.dt.float32
DTYPES = {
    "float16": (mybir.dt.float16, np.float16),
    "bfloat16": (mybir.dt.bfloat16, None),  # np dtype resolved lazily
}
N_CORES = 8
SHIFTS = [(0, 0), (1, 0), (0, 1), (1, 1)]

NG = 128           # row groups per image (X=1024 / 8)
YG = 128           # col groups per image
MROW = 7 * YG + 1  # packed mask row length (896 live + zero pad col)
MIMG = 897 * MROW  # elements per packed mask image ((896+1 pad row) * 897)
EROW = 6 * YG      # packed edge row length (6 live col classes)
ENROW = 7 * NG     # edge rows per image (7 slots per group, dead = zero)
EIMG = ENROW * EROW


def _np_dtype(name):
    if name == "float16":
        return np.float16
    import ml_dtypes

    return ml_dtypes.bfloat16


def _geom(idx):
    """Keep-classes and live/dead slots for mask_index idx."""
    sx, sy = SHIFTS[idx]
    xdrop = (7 - 4 * sx) % 8
    ydrop = (7 - 4 * sy) % 8
    KR = [c for c in range(8) if c != xdrop]          # mask row classes kept
    KC = [c for c in range(8) if c != ydrop]          # mask col classes kept
    dead_x = {(6 - 4 * sx) % 8, (7 - 4 * sx) % 8}     # term row classes dead
    dead_y = {(6 - 4 * sy) % 8, (7 - 4 * sy) % 8}
    live_s = [i for i, c in enumerate(KR) if c not in dead_x]  # 6 slots
    live_j = [i for i, c in enumerate(KC) if c not in dead_y]  # 6 slots
    # live mask cols contiguous (sy==0) -> 6-wide edge col groups; else
    # (sy==1) 7-wide groups with the dead-term column zeroed
    WJ = 6 if live_j == list(range(6)) else 7
    return KR, KC, live_s, live_j, WJ


def _build_program(Bc: int, idx: int, niter: int = 1, join: str = "hybrid",
                   dt_name: str = "float16"):
    """Per-core program. Inputs: mask [Bc,897,897], edge [Bc,896,768] in
    dt_name. Output: out [1,1] f32 partial sum over this core's images."""
    _, _, _, _, WJ = _geom(idx)
    DT = DTYPES[dt_name][0]
    WS = 7
    erow = WJ * YG    # edge row length for this idx
    eimg = ENROW * erow
    N = WS * YG * WJ  # free elems per partition per image
    K = Bc            # accum column per image

    nc = bacc.Bacc("TRN2", target_bir_lowering=False, debug=False)
    mask_h = nc.dram_tensor("mask", [Bc, 897, MROW], DT, kind="ExternalInput")
    edge_h = nc.dram_tensor("edge", [Bc, ENROW, erow], DT, kind="ExternalInput")
    out_h = nc.dram_tensor("out", [1, 1], F32, kind="ExternalOutput")

    with tile.TileContext(nc) as tc:
        with (
            tc.tile_pool(name="mt", bufs=2) as mt_pool,
            tc.tile_pool(name="et", bufs=2) as et_pool,
            tc.tile_pool(name="wa", bufs=2) as wa_pool,
            tc.tile_pool(name="wb", bufs=1 if (join == "act" or WJ == 7) else 2) as wb_pool,
            tc.tile_pool(name="psum", bufs=1, space="PSUM") as ps_pool,
            tc.tile_pool(name="const", bufs=1) as c_pool,
        ):
            ones_t = c_pool.tile([128, 1], F32)
            acc_p2 = c_pool.tile([128, 2 * K], F32)
            acc_t1 = c_pool.tile([128, K], F32)
            nc.vector.memset(ones_t[:], 1.0)
            nc.vector.memset(acc_p2[:], 0.0)
            if join != "ttr":
                nc.vector.memset(acc_t1[:], 0.0)

            # mask DMA is split so the first products start ~4us earlier
            # and image b+1's products fill the DVE gap while image b's
            # squares run on ACT.  The dead term slot (sx!=0 only) sits
            # between the live ranges: products skip it entirely -- its
            # ax0/ay0 slices are zeroed once per buffer during the DMA
            # ramp (on ACT, hidden) so downstream full-slot ops see exact
            # zeros, and the chunk DMAs then need no overlap row.
            _, _, live_s_l, _, _ = _geom(idx)
            dead_s = [s for s in range(WS) if s not in live_s_l]
            # contiguous live ranges
            rngs = []
            for s in live_s_l:
                if rngs and s == rngs[-1][0] + rngs[-1][1]:
                    rngs[-1] = (rngs[-1][0], rngs[-1][1] + 1)
                else:
                    rngs.append((s, 1))
            # chunk c covers term slots [s0, s0+w): mask rows s0..s0+w.
            # image 0's chunks are subdivided to <=2 slots so the DMA ring
            # paces DVE's products gaplessly during the ramp; later images
            # (mask resident before use) keep whole ranges.
            chunksN = [(s0, w, s0, w + 1) for (s0, w) in rngs]
            chunks0 = []
            for (s0, w) in rngs:
                o = 0
                while o < w:
                    ww = min(2, w - o)
                    chunks0.append((s0 + o, ww, s0 + o, ww + 1))
                    o += ww

            def emit_iter():
                # all input DMAs ride the sync HWDGE ring (FIFO per engine)
                # in exact consumption order: mask chunks for every image
                # first, edges after -- edge isn't needed until the join.
                r_lo = chunksN[0][2]
                r_hi = chunksN[-1][2] + chunksN[-1][3]
                loads = []
                for b in range(Bc):
                    mcs = []
                    if b == 0:
                        for ci, (_, _, r0, nr) in enumerate(chunks0):
                            mc = mt_pool.tile(
                                [128, nr * MROW], DT, tag=f"mc{ci}_0", bufs=1
                            )
                            nc.sync.dma_start(
                                mc[:],
                                AP(mask_h, b * MIMG + r0 * MROW,
                                   [[7 * MROW, 128], [1, nr * MROW]]),
                            )
                            mcs.append((mc, r0))
                    else:
                        nr = r_hi - r_lo
                        mc = mt_pool.tile(
                            [128, nr * MROW], DT, tag=f"mcw_{b % 2}", bufs=1
                        )
                        nc.sync.dma_start(
                            mc[:],
                            AP(mask_h, b * MIMG + r_lo * MROW,
                               [[7 * MROW, 128], [1, nr * MROW]]),
                        )
                        mcs = [(mc, r_lo)] * len(chunksN)
                    loads.append(mcs)
                ets = []
                for b in range(Bc):
                    et_t = et_pool.tile([128, WS * erow], DT, tag=f"et_{b % 2}", bufs=1)
                    nc.sync.dma_start(
                        et_t[:],
                        AP(edge_h, b * eimg, [[WS * erow, 128], [1, WS * erow]]),
                    )
                    ets.append(et_t)
                # pre-allocate product tiles and zero the dead-slot
                # slices once per buffer while DVE/ACT idle during the ramp
                prods = []
                for b in range(Bc):
                    shape = [128, WS, YG, WJ]
                    ax0 = wa_pool.tile(shape, DT, tag=f"ax0_{b % 2}", bufs=1)
                    ay0 = wa_pool.tile(shape, DT, tag=f"ay0_{b % 2}", bufs=1)
                    for s in dead_s:
                        blk = YG * WJ
                        for t in (ax0, ay0):
                            fv = t[:].rearrange("p a b c -> p (a b c)")
                            nc.vector.memset(fv[:, s * blk : (s + 1) * blk], 0.0)
                    prods.append((ax0, ay0))

                for b in range(Bc):
                    mcs = loads[b]
                    et_t = ets[b]
                    e_flat = et_t[:]
                    ax0, ay0 = prods[b]

                    bchunks = chunks0 if b == 0 else chunksN
                    for (mc, base), (s0, w, r0, nr) in zip(mcs, bchunks):
                        nsl = mc.shape[1] // MROW
                        coff = r0 - base  # chunk-local first row
                        mv = mc[:].rearrange("p (s y) -> p s y", s=nsl)

                        def mview(srow):
                            sr = coff + srow
                            return (
                                mv[:, sr : sr + w, 0 : 7 * YG]
                                .rearrange("p s (g j) -> p s g j", j=7)
                                [:, :, :, 0:WJ]
                            )

                        # col+1 via flat offset so the last group's j+1
                        # lands in the zero pad col at flat position 896
                        vv = mv[:, coff : coff + w, 1 : 1 + 7 * YG]
                        vyc = vv.rearrange("p s (g j) -> p s g j", j=7)[
                            :, :, :, 0:WJ
                        ]
                        nc.vector.tensor_mul(ay0[:, s0 : s0 + w], mview(0), vyc)
                        nc.vector.tensor_mul(
                            ax0[:, s0 : s0 + w], mview(0), mview(1)
                        )
                    sqx = wb_pool.tile([128, N], DT)
                    sqy = wb_pool.tile([128, N], DT)
                    flat = lambda t: t[:].rearrange("p a b c -> p (a b c)")
                    nc.scalar.activation(
                        sqy[:], flat(ay0),
                        mybir.ActivationFunctionType.Square,
                        bias=1.0, scale=-1.0,
                    )
                    nc.scalar.activation(
                        sqx[:], flat(ax0),
                        mybir.ActivationFunctionType.Square,
                        bias=1.0, scale=-1.0,
                    )

                    t1 = wb_pool.tile([128, N], DT)
                    p2 = None if join == "hybrid" else wb_pool.tile([128, N], DT)
                    if join == "hybrid":
                        # t1 = (sqy-1)*e via 4x TS + 2x TT; the final image
                        # joins via STT+accum (in-op, no tail), earlier
                        # images via TT + ACT Copy accum (overlaps later
                        # images' DVE work).  p2 reuses dead buffers (sqy /
                        # qy) so two images fit in SBUF with bufs=2.
                        qy = wb_pool.tile([128, N], DT)
                        nc.vector.tensor_scalar(
                            qy[:], sqy[:], 1.0, None,
                            op0=mybir.AluOpType.subtract,
                        )
                        nc.vector.tensor_mul(t1[:], qy[:], e_flat)
                        if b < Bc - 1:
                            qx = wb_pool.tile([128, N], DT)
                            nc.vector.tensor_scalar(
                                qx[:], sqx[:], 1.0, None,
                                op0=mybir.AluOpType.subtract,
                            )
                            nc.vector.tensor_mul(sqy[:], qx[:], t1[:])
                            nc.scalar.activation(
                                qx[:], sqy[:],
                                mybir.ActivationFunctionType.Copy,
                                accum_out=acc_p2[:, 2 * b : 2 * b + 1],
                            )
                        else:
                            # final image: slice the 1x STT (and its own
                            # accum column) per live range -- the skipped
                            # dead-slot terms are exactly zero
                            sqx4 = sqx[:].rearrange("p (s z) -> p s z", s=WS)
                            t14 = t1[:].rearrange("p (s z) -> p s z", s=WS)
                            qy4 = qy[:].rearrange("p (s z) -> p s z", s=WS)
                            for ri, (s0, w) in enumerate(rngs):
                                nc.vector.scalar_tensor_tensor(
                                    qy4[:, s0 : s0 + w],
                                    sqx4[:, s0 : s0 + w], 1.0,
                                    t14[:, s0 : s0 + w],
                                    op0=mybir.AluOpType.subtract,
                                    op1=mybir.AluOpType.mult,
                                    accum_out=acc_p2[:, 2 * b + ri : 2 * b + ri + 1],
                                )
                    elif join == "stt":
                        nc.vector.scalar_tensor_tensor(
                            t1[:], sqy[:], 1.0, e_flat,
                            op0=mybir.AluOpType.subtract,
                            op1=mybir.AluOpType.mult,
                        )
                        nc.vector.scalar_tensor_tensor(
                            p2[:], sqx[:], 1.0, t1[:],
                            op0=mybir.AluOpType.subtract,
                            op1=mybir.AluOpType.mult,
                            accum_out=acc_p2[:, b : b + 1],
                        )
                    elif join == "ttr":
                        nc.vector.scalar_tensor_tensor(
                            t1[:], sqy[:], 1.0, e_flat,
                            op0=mybir.AluOpType.subtract,
                            op1=mybir.AluOpType.mult,
                            accum_out=acc_t1[:, b : b + 1],
                        )
                        dummy = wb_pool.tile([128, 1], DT)
                        nc.vector.tensor_tensor_reduce(
                            dummy[:].broadcast_to((128, N)),
                            sqx[:], t1[:],
                            scale=1.0, scalar=0.0,
                            op0=mybir.AluOpType.mult,
                            op1=mybir.AluOpType.add,
                            accum_out=acc_p2[:, b : b + 1],
                        )
                    elif join == "act":
                        qy = wb_pool.tile([128, N], DT)
                        qx = wb_pool.tile([128, N], DT)
                        nc.vector.tensor_scalar(
                            qy[:], sqy[:], 1.0, None,
                            op0=mybir.AluOpType.subtract,
                        )
                        nc.vector.tensor_mul(t1[:], qy[:], e_flat)
                        nc.vector.tensor_scalar(
                            qx[:], sqx[:], 1.0, None,
                            op0=mybir.AluOpType.subtract,
                        )
                        nc.vector.tensor_mul(p2[:], qx[:], t1[:])
                        nc.scalar.activation(
                            t1[:], p2[:],
                            mybir.ActivationFunctionType.Copy,
                            accum_out=acc_p2[:, b : b + 1],
                        )
                    else:
                        raise ValueError(join)

                # total; ttr accumulated sum(sqx*t1) so subtract sum(t1)
                red_p = c_pool.tile([128, 1], F32)
                red_t = c_pool.tile([128, 1], F32)
                nc.vector.reduce_sum(red_p[:], acc_p2[:], axis=mybir.AxisListType.X)
                nc.vector.reduce_sum(red_t[:], acc_t1[:], axis=mybir.AxisListType.X)
                nc.vector.tensor_sub(red_p[:], red_p[:], red_t[:])
                out_ps = ps_pool.tile([1, 1], F32)
                nc.tensor.matmul(out_ps[:], red_p[:], ones_t[:], start=True, stop=True)
                out_sb = c_pool.tile([1, 1], F32)
                nc.vector.tensor_copy(out_sb[:], out_ps[:])
                nc.sync.dma_start(out_h.ap(), out_sb[:])

            if niter == 1:
                emit_iter()
            else:
                with tc.For_i(0, niter, 1):
                    emit_iter()

    nc.compile()
    return nc


def _pack_host(mask, edge, idx, dt_name="float16"):
    """Pack f32 [B,X,Y,{4,1}] inputs to the device layouts."""
    npdt = _np_dtype(dt_name)
    B, X, Y, _ = mask.shape
    KR, KC, live_s, live_j, WJ = _geom(idx)
    erow = WJ * YG
    m = mask[..., idx].reshape(B, NG, 8, Y)[:, :, KR, :]
    m = m.reshape(B, 7 * NG, YG, 8)[..., KC]
    mdev = np.zeros((B, 897, MROW), npdt)
    mdev[:, :896, :896] = m.reshape(B, 896, 896).astype(npdt)
    # edge: rows 7g+s aligned with mask slot s; dead slots stay zero.
    # col slot jj maps to orig class KC[jj] (WJ==7) or KC[live_j[jj]]
    # (WJ==6); dead-term columns stay zero.
    e = edge[..., 0]
    EC = list(range(7)) if WJ == 7 else live_j
    dead_cols = [jj for jj in range(WJ) if EC[jj] not in live_j]
    edev = np.zeros((B, NG, 7, erow), npdt)
    for s in live_s:
        c = KR[s]
        es = e.reshape(B, NG, 8, Y)[:, :, c, :].reshape(B, NG, YG, 8)
        es = es[..., [KC[jj] for jj in EC]].astype(npdt)
        if dead_cols:
            es[..., dead_cols] = 0
        edev[:, :, s, :] = es.reshape(B, NG, erow)
    return mdev, np.ascontiguousarray(edev.reshape(B, ENROW, erow))


def _run(mask, edge, loss_old, idx, trace=False, niter=1, join="hybrid",
         dt_name=None):
    if dt_name is None:
        dt_name = os.environ.get("KDT", "float16")
    B, X, Y, _ = mask.shape
    assert B % N_CORES == 0
    Bc = B // N_CORES

    nc = _build_program(Bc, idx, niter=niter, join=join, dt_name=dt_name)
    mdev, edev = _pack_host(mask, edge, idx, dt_name)
    in_maps = [
        {
            "mask": mdev[i * Bc : (i + 1) * Bc],
            "edge": edev[i * Bc : (i + 1) * Bc],
        }
        for i in range(N_CORES)
    ]
    res = run_bass_kernel_spmd(nc, in_maps, list(range(N_CORES)), trace=trace)
    total = float(sum(float(res.results[i]["out"][0, 0]) for i in range(N_CORES)))
    n_patch = ((X + 8) // 8) * ((Y + 8) // 8)
    out = np.float32(np.asarray(loss_old, dtype=np.float32) + total / (B * n_patch))
    return np.asarray(out, dtype=np.float32), res


def kernel(resized_image, mask_combined, edge_map, loss_old, mask_index):
    mask = np.ascontiguousarray(np.asarray(mask_combined, dtype=np.float32))
    edge = np.ascontiguousarray(np.asarray(edge_map, dtype=np.float32))
    idx = int(np.asarray(mask_index))
    out, _ = _run(mask, edge, loss_old, idx)
    return out


# revision 30
# speedup vs baseline: 1.0426x; 1.0426x over previous
"""Trainium2 Bass kernel for the supervoxel erode/edge loss module.

The reference pads a [B,X,Y] grid (offset 4*sx rows / 4*sy cols), tiles it
into 8x8 patches, zeroes each patch's last row/col of the mask channel,
erodes along both patch axes and sums eroded*edge.  The erode
`a*b + (1-a)*a + (1-b)*a` equals `2a - a^2 = 1 - (1-a)^2` with
a = m(i)*m(i+1), so the whole module collapses to a global elementwise
expression on the unpadded grid (validated to f64 exactness):

    mt(x,y) = mask[b,x,y,idx] * [(x+4sx)%8 != 7] * [(y+4sy)%8 != 7]
    ax = mt(x,y)*mt(x+1,y); ay = mt(x,y)*mt(x,y+1)   (zero past image edge)
    total = sum (1-(1-ax)^2) * (1-(1-ay)^2) * edge
    out = loss_old + total / (B * ((X+8)//8) * ((Y+8)//8))

Host-side packing (layout/precision only -- all arithmetic on device):
  * only mask channel `idx` matters (4x traffic cut), and mask rows with
    x%8 == 7-4sx / cols with y%8 == 7-4sy are provably dead, as are edge
    rows x%8 in {6-4sx,7-4sx} and cols y%8 in {6-4sy,7-4sy}.
  * f32 -> f16: inputs are U[0,1) and the loss is a mean of ~10M terms;
    measured end-to-end rel err ~5e-6 (gate is 2e-2).
  mask -> [Bc, 897, 897] f16: 7 live row/col classes per group of 8, plus
    one zero pad row and col (exact zero shift-ins at image edges).
  edge -> [Bc, 896, 768] f16 (for sy==0): 7 row slots per group aligned
    with the mask slots -- dead-term slots are zero rows -- by 6 live col
    classes.  For sy==1 cols are 7-wide with the dead column zeroed.

Device layout: partition p = row-group p (128 groups per image).  Each
partition holds 8 consecutive packed mask rows (7 + 1 overlap row = the
next group's first row), so a whole image's mask is 128 contiguous ~11KiB
DMA descriptors, split into per-live-range chunks (rows 0:3 / 3:8 for
sx=1, no rows read twice) so compute starts as soon as the first chunk
lands.  Products are emitted only for live term slots; the dead slot's
ax0/ay0 slices are zeroed once during the DMA ramp so the downstream
full-slot ops see exact zeros.  ALL input DMAs ride the sync
HWDGE ring (FIFO per engine) in exact consumption order -- mask chunks
for every image first, then edges (only needed by the join) -- so the
mask stream is never starved by edge traffic.  Both neighbor shifts are
free-dim views; no PE/transpose work at all:

    v = tile[:, 0:7, g, 0:6]   vx = tile[:, 1:8, g, 0:6]   vy = v at col+1

Engine assignment (measured on HW): GpSimd is avoided entirely -- any
GpSimd op locks the shared SBUF port pair that DVE needs for its second
tensor operand and fully blocks it.  scalar_tensor_tensor runs at 1x
(no 2x uop) so it appears exactly once, as the final op, where its free
in-op accum_out beats a separate pass; everything else is 2x/4x DVE or
ACT.  Per image, emitted y-chain first (the join consumes sqy first):

    DVE : ay0 = v*vy; ax0 = v*vx          (tensor_tensor, 2x f16)
    ACT : sqy = (1-ay0)^2; sqx = (1-ax0)^2  (Square activation)
    DVE : qy = sqy-1 (tensor_scalar 4x); t1 = qy*e (2x)
    last image : p2 = (sqx-1)*t1 [STT 1x, accum_out = the partial sum]
    earlier    : qx = sqx-1 (4x); p2 = qx*t1 (2x); sum via ACT Copy
                 accum_out (overlaps later images' DVE work)
    (sign flips cancel: (sqx-1)(sqy-1) = (1-sqx)(1-sqy))

A tiny reduce + PE ones-matmul then yields the scalar partial sum.

Sharding: data-parallel over batch, B/8 images per core on 8 cores; the
per-core partial sums combine on host (the mean needs no collective).

Progression on HW (exec_time, core 0): 354.8us (f32 tiled baseline) ->
61.1us (packed-f16 rewrite) -> 57.0us (hybrid join, no GpSimd) ->
53.4us (single-ring DMA ordering) -> 51.5us (y-first) -> 49.8us (SC=3)
-> 48.8us (dead-slot products skipped; chunk DMAs with no overlap row)
-> 48.7us (single mask DMA for later images; final STT sliced per range).
"""

import os
import sys

sys.path.insert(0, "/opt/trn_rl_repo")

import numpy as np

from concourse import bacc, bass, mybir, tile
from concourse.ap import AP
from concourse.bass_utils import run_bass_kernel_spmd

F32 = mybir.dt.float32
DTYPES = {
    "float16": (mybir.dt.float16, np.float16),
    "bfloat16": (mybir.dt.bfloat16, None),  # np dtype resolved lazily
}
N_CORES = 8
SHIFTS = [(0, 0), (1, 0), (0, 1), (1, 1)]

NG = 128           # row groups per image (X=1024 / 8)
YG = 128           # col groups per image
MROW = 7 * YG + 1  # packed mask row length (896 live + zero pad col)
MIMG = 897 * MROW  # elements per packed mask image ((896+1 pad row) * 897)
EROW = 6 * YG      # packed edge row length (6 live col classes)
ENROW = 7 * NG     # edge rows per image (7 slots per group, dead = zero)
EIMG = ENROW * EROW


def _np_dtype(name):
    if name == "float16":
        return np.float16
    import ml_dtypes

    return ml_dtypes.bfloat16


def _geom(idx):
    """Keep-classes and live/dead slots for mask_index idx."""
    sx, sy = SHIFTS[idx]
    xdrop = (7 - 4 * sx) % 8
    ydrop = (7 - 4 * sy) % 8
    KR = [c for c in range(8) if c != xdrop]          # mask row classes kept
    KC = [c for c in range(8) if c != ydrop]          # mask col classes kept
    dead_x = {(6 - 4 * sx) % 8, (7 - 4 * sx) % 8}     # term row classes dead
    dead_y = {(6 - 4 * sy) % 8, (7 - 4 * sy) % 8}
    live_s = [i for i, c in enumerate(KR) if c not in dead_x]  # 6 slots
    live_j = [i for i, c in enumerate(KC) if c not in dead_y]  # 6 slots
    # live mask cols contiguous (sy==0) -> 6-wide edge col groups; else
    # (sy==1) 7-wide groups with the dead-term column zeroed
    WJ = 6 if live_j == list(range(6)) else 7
    return KR, KC, live_s, live_j, WJ


def _build_program(Bc: int, idx: int, niter: int = 1, join: str = "hybrid",
                   dt_name: str = "float16"):
    """Per-core program. Inputs: mask [Bc,897,897], edge [Bc,896,768] in
    dt_name. Output: out [1,1] f32 partial sum over this core's images."""
    _, _, _, _, WJ = _geom(idx)
    DT = DTYPES[dt_name][0]
    WS = 7
    erow = WJ * YG    # edge row length for this idx
    eimg = ENROW * erow
    N = WS * YG * WJ  # free elems per partition per image
    K = Bc            # accum column per image

    nc = bacc.Bacc("TRN2", target_bir_lowering=False, debug=False)
    mask_h = nc.dram_tensor("mask", [Bc, 897, MROW], DT, kind="ExternalInput")
    edge_h = nc.dram_tensor("edge", [Bc, ENROW, erow], DT, kind="ExternalInput")
    out_h = nc.dram_tensor("out", [1, 1], F32, kind="ExternalOutput")

    with tile.TileContext(nc) as tc:
        with (
            tc.tile_pool(name="mt", bufs=2) as mt_pool,
            tc.tile_pool(name="et", bufs=2) as et_pool,
            tc.tile_pool(name="wa", bufs=2) as wa_pool,
            tc.tile_pool(name="wb", bufs=1 if (join == "act" or WJ == 7) else 2) as wb_pool,
            tc.tile_pool(name="psum", bufs=1, space="PSUM") as ps_pool,
            tc.tile_pool(name="const", bufs=1) as c_pool,
        ):
            ones_t = c_pool.tile([128, 1], F32)
            acc_p2 = c_pool.tile([128, 2 * K], F32)
            acc_t1 = c_pool.tile([128, K], F32)
            nc.vector.memset(ones_t[:], 1.0)
            nc.vector.memset(acc_p2[:], 0.0)
            if join != "ttr":
                nc.vector.memset(acc_t1[:], 0.0)

            # mask DMA is split so the first products start ~4us earlier
            # and image b+1's products fill the DVE gap while image b's
            # squares run on ACT.  The dead term slot (sx!=0 only) sits
            # between the live ranges: products skip it entirely -- its
            # ax0/ay0 slices are zeroed once per buffer during the DMA
            # ramp (on ACT, hidden) so downstream full-slot ops see exact
            # zeros, and the chunk DMAs then need no overlap row.
            _, _, live_s_l, _, _ = _geom(idx)
            dead_s = [s for s in range(WS) if s not in live_s_l]
            # contiguous live ranges
            rngs = []
            for s in live_s_l:
                if rngs and s == rngs[-1][0] + rngs[-1][1]:
                    rngs[-1] = (rngs[-1][0], rngs[-1][1] + 1)
                else:
                    rngs.append((s, 1))
            # chunk c covers term slots [s0, s0+w): mask rows s0..s0+w
            chunks = [(s0, w, s0, w + 1) for (s0, w) in rngs]

            def emit_iter():
                # all input DMAs ride the sync HWDGE ring (FIFO per engine)
                # in exact consumption order: mask chunks for every image
                # first, edges after -- edge isn't needed until the join.
                r_lo = chunks[0][2]
                r_hi = chunks[-1][2] + chunks[-1][3]
                loads = []
                for b in range(Bc):
                    mcs = []
                    if b == 0:
                        for ci, (_, _, r0, nr) in enumerate(chunks):
                            mc = mt_pool.tile(
                                [128, nr * MROW], DT, tag=f"mc{ci}_0", bufs=1
                            )
                            nc.sync.dma_start(
                                mc[:],
                                AP(mask_h, b * MIMG + r0 * MROW,
                                   [[7 * MROW, 128], [1, nr * MROW]]),
                            )
                            mcs.append((mc, r0))
                    else:
                        nr = r_hi - r_lo
                        mc = mt_pool.tile(
                            [128, nr * MROW], DT, tag=f"mcw_{b % 2}", bufs=1
                        )
                        nc.sync.dma_start(
                            mc[:],
                            AP(mask_h, b * MIMG + r_lo * MROW,
                               [[7 * MROW, 128], [1, nr * MROW]]),
                        )
                        mcs = [(mc, r_lo)] * len(chunks)
                    loads.append(mcs)
                ets = []
                for b in range(Bc):
                    et_t = et_pool.tile([128, WS * erow], DT, tag=f"et_{b % 2}", bufs=1)
                    nc.sync.dma_start(
                        et_t[:],
                        AP(edge_h, b * eimg, [[WS * erow, 128], [1, WS * erow]]),
                    )
                    ets.append(et_t)
                # pre-allocate product tiles and zero the dead-slot
                # slices once per buffer while DVE/ACT idle during the ramp
                prods = []
                for b in range(Bc):
                    shape = [128, WS, YG, WJ]
                    ax0 = wa_pool.tile(shape, DT, tag=f"ax0_{b % 2}", bufs=1)
                    ay0 = wa_pool.tile(shape, DT, tag=f"ay0_{b % 2}", bufs=1)
                    for s in dead_s:
                        blk = YG * WJ
                        for t in (ax0, ay0):
                            fv = t[:].rearrange("p a b c -> p (a b c)")
                            nc.vector.memset(fv[:, s * blk : (s + 1) * blk], 0.0)
                    prods.append((ax0, ay0))

                for b in range(Bc):
                    mcs = loads[b]
                    et_t = ets[b]
                    e_flat = et_t[:]
                    ax0, ay0 = prods[b]

                    for (mc, base), (s0, w, r0, nr) in zip(mcs, chunks):
                        nsl = mc.shape[1] // MROW
                        coff = r0 - base  # chunk-local first row
                        mv = mc[:].rearrange("p (s y) -> p s y", s=nsl)

                        def mview(srow):
                            sr = coff + srow
                            return (
                                mv[:, sr : sr + w, 0 : 7 * YG]
                                .rearrange("p s (g j) -> p s g j", j=7)
                                [:, :, :, 0:WJ]
                            )

                        # col+1 via flat offset so the last group's j+1
                        # lands in the zero pad col at flat position 896
                        vv = mv[:, coff : coff + w, 1 : 1 + 7 * YG]
                        vyc = vv.rearrange("p s (g j) -> p s g j", j=7)[
                            :, :, :, 0:WJ
                        ]
                        nc.vector.tensor_mul(ay0[:, s0 : s0 + w], mview(0), vyc)
                        nc.vector.tensor_mul(
                            ax0[:, s0 : s0 + w], mview(0), mview(1)
                        )
                    sqx = wb_pool.tile([128, N], DT)
                    sqy = wb_pool.tile([128, N], DT)
                    flat = lambda t: t[:].rearrange("p a b c -> p (a b c)")
                    nc.scalar.activation(
                        sqy[:], flat(ay0),
                        mybir.ActivationFunctionType.Square,
                        bias=1.0, scale=-1.0,
                    )
                    nc.scalar.activation(
                        sqx[:], flat(ax0),
                        mybir.ActivationFunctionType.Square,
                        bias=1.0, scale=-1.0,
                    )

                    t1 = wb_pool.tile([128, N], DT)
                    p2 = None if join == "hybrid" else wb_pool.tile([128, N], DT)
                    if join == "hybrid":
                        # t1 = (sqy-1)*e via 4x TS + 2x TT; the final image
                        # joins via STT+accum (in-op, no tail), earlier
                        # images via TT + ACT Copy accum (overlaps later
                        # images' DVE work).  p2 reuses dead buffers (sqy /
                        # qy) so two images fit in SBUF with bufs=2.
                        qy = wb_pool.tile([128, N], DT)
                        nc.vector.tensor_scalar(
                            qy[:], sqy[:], 1.0, None,
                            op0=mybir.AluOpType.subtract,
                        )
                        nc.vector.tensor_mul(t1[:], qy[:], e_flat)
                        if b < Bc - 1:
                            qx = wb_pool.tile([128, N], DT)
                            nc.vector.tensor_scalar(
                                qx[:], sqx[:], 1.0, None,
                                op0=mybir.AluOpType.subtract,
                            )
                            nc.vector.tensor_mul(sqy[:], qx[:], t1[:])
                            nc.scalar.activation(
                                qx[:], sqy[:],
                                mybir.ActivationFunctionType.Copy,
                                accum_out=acc_p2[:, 2 * b : 2 * b + 1],
                            )
                        else:
                            # final image: slice the 1x STT (and its own
                            # accum column) per live range -- the skipped
                            # dead-slot terms are exactly zero
                            sqx4 = sqx[:].rearrange("p (s z) -> p s z", s=WS)
                            t14 = t1[:].rearrange("p (s z) -> p s z", s=WS)
                            qy4 = qy[:].rearrange("p (s z) -> p s z", s=WS)
                            for ri, (s0, w) in enumerate(rngs):
                                nc.vector.scalar_tensor_tensor(
                                    qy4[:, s0 : s0 + w],
                                    sqx4[:, s0 : s0 + w], 1.0,
                                    t14[:, s0 : s0 + w],
                                    op0=mybir.AluOpType.subtract,
                                    op1=mybir.AluOpType.mult,
                                    accum_out=acc_p2[:, 2 * b + ri : 2 * b + ri + 1],
                                )
                    elif join == "stt":
                        nc.vector.scalar_tensor_tensor(
                            t1[:], sqy[:], 1.0, e_flat,
                            op0=mybir.AluOpType.subtract,
                            op1=mybir.AluOpType.mult,
                        )
                        nc.vector.scalar_tensor_tensor(
                            p2[:], sqx[:], 1.0, t1[:],
                            op0=mybir.AluOpType.subtract,
                            op1=mybir.AluOpType.mult,
                            accum_out=acc_p2[:, b : b + 1],
                        )
                    elif join == "ttr":
                        nc.vector.scalar_tensor_tensor(
                            t1[:], sqy[:], 1.0, e_flat,
                            op0=mybir.AluOpType.subtract,
                            op1=mybir.AluOpType.mult,
                            accum_out=acc_t1[:, b : b + 1],
                        )
                        dummy = wb_pool.tile([128, 1], DT)
                        nc.vector.tensor_tensor_reduce(
                            dummy[:].broadcast_to((128, N)),
                            sqx[:], t1[:],
                            scale=1.0, scalar=0.0,
                            op0=mybir.AluOpType.mult,
                            op1=mybir.AluOpType.add,
                            accum_out=acc_p2[:, b : b + 1],
                        )
                    elif join == "act":
                        qy = wb_pool.tile([128, N], DT)
                        qx = wb_pool.tile([128, N], DT)
                        nc.vector.tensor_scalar(
                            qy[:], sqy[:], 1.0, None,
                            op0=mybir.AluOpType.subtract,
                        )
                        nc.vector.tensor_mul(t1[:], qy[:], e_flat)
                        nc.vector.tensor_scalar(
                            qx[:], sqx[:], 1.0, None,
                            op0=mybir.AluOpType.subtract,
                        )
                        nc.vector.tensor_mul(p2[:], qx[:], t1[:])
                        nc.scalar.activation(
                            t1[:], p2[:],
                            mybir.ActivationFunctionType.Copy,
                            accum_out=acc_p2[:, b : b + 1],
                        )
                    else:
                        raise ValueError(join)

                # total; ttr accumulated sum(sqx*t1) so subtract sum(t1)
                red_p = c_pool.tile([128, 1], F32)
                red_t = c_pool.tile([128, 1], F32)
                nc.vector.reduce_sum(red_p[:], acc_p2[:], axis=mybir.AxisListType.X)
                nc.vector.reduce_sum(red_t[:], acc_t1[:], axis=mybir.AxisListType.X)
                nc.vector.tensor_sub(red_p[:], red_p[:], red_t[:])
                out_ps = ps_pool.tile([1, 1], F32)
                nc.tensor.matmul(out_ps[:], red_p[:], ones_t[:], start=True, stop=True)
                out_sb = c_pool.tile([1, 1], F32)
                nc.vector.tensor_copy(out_sb[:], out_ps[:])
                nc.sync.dma_start(out_h.ap(), out_sb[:])

            if niter == 1:
                emit_iter()
            else:
                with tc.For_i(0, niter, 1):
                    emit_iter()

    nc.compile()
    return nc


def _pack_host(mask, edge, idx, dt_name="float16"):
    """Pack f32 [B,X,Y,{4,1}] inputs to the device layouts."""
    npdt = _np_dtype(dt_name)
    B, X, Y, _ = mask.shape
    KR, KC, live_s, live_j, WJ = _geom(idx)
    erow = WJ * YG
    m = mask[..., idx].reshape(B, NG, 8, Y)[:, :, KR, :]
    m = m.reshape(B, 7 * NG, YG, 8)[..., KC]
    mdev = np.zeros((B, 897, MROW), npdt)
    mdev[:, :896, :896] = m.reshape(B, 896, 896).astype(npdt)
    # edge: rows 7g+s aligned with mask slot s; dead slots stay zero.
    # col slot jj maps to orig class KC[jj] (WJ==7) or KC[live_j[jj]]
    # (WJ==6); dead-term columns stay zero.
    e = edge[..., 0]
    EC = list(range(7)) if WJ == 7 else live_j
    dead_cols = [jj for jj in range(WJ) if EC[jj] not in live_j]
    edev = np.zeros((B, NG, 7, erow), npdt)
    for s in live_s:
        c = KR[s]
        es = e.reshape(B, NG, 8, Y)[:, :, c, :].reshape(B, NG, YG, 8)
        es = es[..., [KC[jj] for jj in EC]].astype(npdt)
        if dead_cols:
            es[..., dead_cols] = 0
        edev[:, :, s, :] = es.reshape(B, NG, erow)
    return mdev, np.ascontiguousarray(edev.reshape(B, ENROW, erow))


def _run(mask, edge, loss_old, idx, trace=False, niter=1, join="hybrid",
         dt_name=None):
    if dt_name is None:
        dt_name = os.environ.get("KDT", "float16")
    B, X, Y, _ = mask.shape
    assert B % N_CORES == 0
    Bc = B // N_CORES

    nc = _build_program(Bc, idx, niter=niter, join=join, dt_name=dt_name)
    mdev, edev = _pack_host(mask, edge, idx, dt_name)
    in_maps = [
        {
            "mask": mdev[i * Bc : (i + 1) * Bc],
            "edge": edev[i * Bc : (i + 1) * Bc],
        }
        for i in range(N_CORES)
    ]
    res = run_bass_kernel_spmd(nc, in_maps, list(range(N_CORES)), trace=trace)
    total = float(sum(float(res.results[i]["out"][0, 0]) for i in range(N_CORES)))
    n_patch = ((X + 8) // 8) * ((Y + 8) // 8)
    out = np.float32(np.asarray(loss_old, dtype=np.float32) + total / (B * n_patch))
    return np.asarray(out, dtype=np.float32), res


def kernel(resized_image, mask_combined, edge_map, loss_old, mask_index):
    mask = np.ascontiguousarray(np.asarray(mask_combined, dtype=np.float32))
    edge = np.ascontiguousarray(np.asarray(edge_map, dtype=np.float32))
    idx = int(np.asarray(mask_index))
    out, _ = _run(mask, edge, loss_old, idx)
    return out


# revision 33
# speedup vs baseline: 1.0489x; 1.0060x over previous
"""Trainium2 Bass kernel for the supervoxel erode/edge loss module.

The reference pads a [B,X,Y] grid (offset 4*sx rows / 4*sy cols), tiles it
into 8x8 patches, zeroes each patch's last row/col of the mask channel,
erodes along both patch axes and sums eroded*edge.  The erode
`a*b + (1-a)*a + (1-b)*a` equals `2a - a^2 = 1 - (1-a)^2` with
a = m(i)*m(i+1), so the whole module collapses to a global elementwise
expression on the unpadded grid (validated to f64 exactness):

    mt(x,y) = mask[b,x,y,idx] * [(x+4sx)%8 != 7] * [(y+4sy)%8 != 7]
    ax = mt(x,y)*mt(x+1,y); ay = mt(x,y)*mt(x,y+1)   (zero past image edge)
    total = sum (1-(1-ax)^2) * (1-(1-ay)^2) * edge
    out = loss_old + total / (B * ((X+8)//8) * ((Y+8)//8))

Host-side packing (layout/precision only -- all arithmetic on device):
  * only mask channel `idx` matters (4x traffic cut), and mask rows with
    x%8 == 7-4sx / cols with y%8 == 7-4sy are provably dead, as are edge
    rows x%8 in {6-4sx,7-4sx} and cols y%8 in {6-4sy,7-4sy}.
  * f32 -> f16: inputs are U[0,1) and the loss is a mean of ~10M terms;
    measured end-to-end rel err ~5e-6 (gate is 2e-2).
  mask -> [Bc, 897, 897] f16: 7 live row/col classes per group of 8, plus
    one zero pad row and col (exact zero shift-ins at image edges).
  edge -> [Bc, 896, 768] f16 (for sy==0): 7 row slots per group aligned
    with the mask slots -- dead-term slots are zero rows -- by 6 live col
    classes.  For sy==1 cols are 7-wide with the dead column zeroed.

Device layout: partition p = row-group p (128 groups per image).  Each
partition holds 8 consecutive packed mask rows (7 + 1 overlap row = the
next group's first row), so a whole image's mask is 128 contiguous ~11KiB
DMA descriptors, split into per-live-range chunks (rows 0:3 / 3:8 for
sx=1, no rows read twice) so compute starts as soon as the first chunk
lands.  Products are emitted only for live term slots; the dead slot's
ax0/ay0 slices are zeroed once during the DMA ramp so the downstream
full-slot ops see exact zeros.  ALL input DMAs ride the sync
HWDGE ring (FIFO per engine) in exact consumption order -- mask chunks
for every image first, then edges (only needed by the join) -- so the
mask stream is never starved by edge traffic.  Both neighbor shifts are
free-dim views; no PE/transpose work at all:

    v = tile[:, 0:7, g, 0:6]   vx = tile[:, 1:8, g, 0:6]   vy = v at col+1

Engine assignment (measured on HW): GpSimd is avoided entirely -- any
GpSimd op locks the shared SBUF port pair that DVE needs for its second
tensor operand and fully blocks it.  scalar_tensor_tensor runs at 1x
(no 2x uop) so it appears exactly once, as the final op, where its free
in-op accum_out beats a separate pass; everything else is 2x/4x DVE or
ACT.  Per image, emitted y-chain first (the join consumes sqy first):

    DVE : ay0 = v*vy; ax0 = v*vx          (tensor_tensor, 2x f16)
    ACT : sqy = (1-ay0)^2; sqx = (1-ax0)^2  (Square activation)
    DVE : qy = sqy-1 (tensor_scalar 4x); t1 = qy*e (2x)
    last image : p2 = (sqx-1)*t1 [STT 1x, accum_out = the partial sum]
    earlier    : qx = sqx-1 (4x); p2 = qx*t1 (2x); sum via ACT Copy
                 accum_out (overlaps later images' DVE work)
    (sign flips cancel: (sqx-1)(sqy-1) = (1-sqx)(1-sqy))

A tiny reduce + PE ones-matmul then yields the scalar partial sum.

Sharding: data-parallel over batch, B/8 images per core on 8 cores; the
per-core partial sums combine on host (the mean needs no collective).

Progression on HW (exec_time, core 0): 354.8us (f32 tiled baseline) ->
61.1us (packed-f16 rewrite) -> 57.0us (hybrid join, no GpSimd) ->
53.4us (single-ring DMA ordering) -> 51.5us (y-first) -> 49.8us (SC=3)
-> 48.8us (dead-slot products skipped; chunk DMAs with no overlap row)
-> 48.7us (single mask DMA for later images; final STT sliced per range).
"""

import os
import sys

sys.path.insert(0, "/opt/trn_rl_repo")

import numpy as np

from concourse import bacc, bass, mybir, tile
from concourse.ap import AP
from concourse.bass_utils import run_bass_kernel_spmd

F32 = mybir.dt.float32
DTYPES = {
    "float16": (mybir.dt.float16, np.float16),
    "bfloat16": (mybir.dt.bfloat16, None),  # np dtype resolved lazily
}
N_CORES = 8
SHIFTS = [(0, 0), (1, 0), (0, 1), (1, 1)]

NG = 128           # row groups per image (X=1024 / 8)
YG = 128           # col groups per image
MROW = 7 * YG + 1  # packed mask row length (896 live + zero pad col)
MIMG = 897 * MROW  # elements per packed mask image ((896+1 pad row) * 897)
EROW = 6 * YG      # packed edge row length (6 live col classes)
ENROW = 7 * NG     # edge rows per image (7 slots per group, dead = zero)
EIMG = ENROW * EROW


def _np_dtype(name):
    if name == "float16":
        return np.float16
    import ml_dtypes

    return ml_dtypes.bfloat16


def _geom(idx):
    """Keep-classes and live/dead slots for mask_index idx."""
    sx, sy = SHIFTS[idx]
    xdrop = (7 - 4 * sx) % 8
    ydrop = (7 - 4 * sy) % 8
    KR = [c for c in range(8) if c != xdrop]          # mask row classes kept
    KC = [c for c in range(8) if c != ydrop]          # mask col classes kept
    dead_x = {(6 - 4 * sx) % 8, (7 - 4 * sx) % 8}     # term row classes dead
    dead_y = {(6 - 4 * sy) % 8, (7 - 4 * sy) % 8}
    live_s = [i for i, c in enumerate(KR) if c not in dead_x]  # 6 slots
    live_j = [i for i, c in enumerate(KC) if c not in dead_y]  # 6 slots
    # live mask cols contiguous (sy==0) -> 6-wide edge col groups; else
    # (sy==1) 7-wide groups with the dead-term column zeroed
    WJ = 6 if live_j == list(range(6)) else 7
    return KR, KC, live_s, live_j, WJ


def _build_program(Bc: int, idx: int, niter: int = 1, join: str = "hybrid",
                   dt_name: str = "float16"):
    """Per-core program. Inputs: mask [Bc,897,897], edge [Bc,896,768] in
    dt_name. Output: out [1,1] f32 partial sum over this core's images."""
    _, _, _, _, WJ = _geom(idx)
    DT = DTYPES[dt_name][0]
    WS = 7
    erow = WJ * YG    # edge row length for this idx
    eimg = ENROW * erow
    N = WS * YG * WJ  # free elems per partition per image
    K = Bc            # accum column per image

    nc = bacc.Bacc("TRN2", target_bir_lowering=False, debug=False)
    mask_h = nc.dram_tensor("mask", [Bc, 897, MROW], DT, kind="ExternalInput")
    edge_h = nc.dram_tensor("edge", [Bc, ENROW, erow], DT, kind="ExternalInput")
    out_h = nc.dram_tensor("out", [1, 1], F32, kind="ExternalOutput")

    with tile.TileContext(nc) as tc:
        with (
            tc.tile_pool(name="mt", bufs=2) as mt_pool,
            tc.tile_pool(name="et", bufs=2) as et_pool,
            tc.tile_pool(name="wa", bufs=2) as wa_pool,
            tc.tile_pool(name="wb", bufs=1 if (join == "act" or WJ == 7) else 2) as wb_pool,
            tc.tile_pool(name="psum", bufs=1, space="PSUM") as ps_pool,
            tc.tile_pool(name="const", bufs=1) as c_pool,
        ):
            ones_t = c_pool.tile([128, 1], F32)
            acc_p2 = c_pool.tile([128, 2 * K], F32)
            acc_t1 = c_pool.tile([128, K], F32)
            nc.vector.memset(ones_t[:], 1.0)
            nc.vector.memset(acc_p2[:], 0.0)

            # mask DMA is split so the first products start ~4us earlier
            # and image b+1's products fill the DVE gap while image b's
            # squares run on ACT.  The dead term slot (sx!=0 only) sits
            # between the live ranges: products skip it entirely -- its
            # ax0/ay0 slices are zeroed once per buffer during the DMA
            # ramp (on ACT, hidden) so downstream full-slot ops see exact
            # zeros, and the chunk DMAs then need no overlap row.
            _, _, live_s_l, _, _ = _geom(idx)
            dead_s = [s for s in range(WS) if s not in live_s_l]
            # contiguous live ranges
            rngs = []
            for s in live_s_l:
                if rngs and s == rngs[-1][0] + rngs[-1][1]:
                    rngs[-1] = (rngs[-1][0], rngs[-1][1] + 1)
                else:
                    rngs.append((s, 1))
            # chunk c covers term slots [s0, s0+w): mask rows s0..s0+w
            chunks = [(s0, w, s0, w + 1) for (s0, w) in rngs]

            def emit_iter():
                # all input DMAs ride the sync HWDGE ring (FIFO per engine)
                # in exact consumption order: mask chunks for every image
                # first, edges after -- edge isn't needed until the join.
                r_lo = chunks[0][2]
                r_hi = chunks[-1][2] + chunks[-1][3]
                loads = []
                for b in range(Bc):
                    mcs = []
                    if b == 0:
                        for ci, (_, _, r0, nr) in enumerate(chunks):
                            mc = mt_pool.tile(
                                [128, nr * MROW], DT, tag=f"mc{ci}_0", bufs=1
                            )
                            nc.sync.dma_start(
                                mc[:],
                                AP(mask_h, b * MIMG + r0 * MROW,
                                   [[7 * MROW, 128], [1, nr * MROW]]),
                            )
                            mcs.append((mc, r0))
                    else:
                        nr = r_hi - r_lo
                        mc = mt_pool.tile(
                            [128, nr * MROW], DT, tag=f"mcw_{b % 2}", bufs=1
                        )
                        nc.sync.dma_start(
                            mc[:],
                            AP(mask_h, b * MIMG + r_lo * MROW,
                               [[7 * MROW, 128], [1, nr * MROW]]),
                        )
                        mcs = [(mc, r_lo)] * len(chunks)
                    loads.append(mcs)
                ets = []
                for b in range(Bc):
                    et_t = et_pool.tile([128, WS * erow], DT, tag=f"et_{b % 2}", bufs=1)
                    nc.sync.dma_start(
                        et_t[:],
                        AP(edge_h, b * eimg, [[WS * erow, 128], [1, WS * erow]]),
                    )
                    ets.append(et_t)
                # pre-allocate product tiles and zero the dead-slot
                # slices once per buffer while DVE/ACT idle during the ramp
                prods = []
                for b in range(Bc):
                    shape = [128, WS, YG, WJ]
                    ax0 = wa_pool.tile(shape, DT, tag=f"ax0_{b % 2}", bufs=1)
                    ay0 = wa_pool.tile(shape, DT, tag=f"ay0_{b % 2}", bufs=1)
                    for s in dead_s:
                        blk = YG * WJ
                        for t in (ax0, ay0):
                            fv = t[:].rearrange("p a b c -> p (a b c)")
                            nc.vector.memset(fv[:, s * blk : (s + 1) * blk], 0.0)
                    prods.append((ax0, ay0))

                for b in range(Bc):
                    mcs = loads[b]
                    et_t = ets[b]
                    e_flat = et_t[:]
                    ax0, ay0 = prods[b]

                    for (mc, base), (s0, w, r0, nr) in zip(mcs, chunks):
                        nsl = mc.shape[1] // MROW
                        coff = r0 - base  # chunk-local first row
                        mv = mc[:].rearrange("p (s y) -> p s y", s=nsl)

                        def mview(srow):
                            sr = coff + srow
                            return (
                                mv[:, sr : sr + w, 0 : 7 * YG]
                                .rearrange("p s (g j) -> p s g j", j=7)
                                [:, :, :, 0:WJ]
                            )

                        # col+1 via flat offset so the last group's j+1
                        # lands in the zero pad col at flat position 896
                        vv = mv[:, coff : coff + w, 1 : 1 + 7 * YG]
                        vyc = vv.rearrange("p s (g j) -> p s g j", j=7)[
                            :, :, :, 0:WJ
                        ]
                        nc.vector.tensor_mul(ay0[:, s0 : s0 + w], mview(0), vyc)
                        nc.vector.tensor_mul(
                            ax0[:, s0 : s0 + w], mview(0), mview(1)
                        )
                    sqx = wb_pool.tile([128, N], DT)
                    sqy = wb_pool.tile([128, N], DT)
                    flat = lambda t: t[:].rearrange("p a b c -> p (a b c)")
                    nc.scalar.activation(
                        sqy[:], flat(ay0),
                        mybir.ActivationFunctionType.Square,
                        bias=1.0, scale=-1.0,
                    )
                    nc.scalar.activation(
                        sqx[:], flat(ax0),
                        mybir.ActivationFunctionType.Square,
                        bias=1.0, scale=-1.0,
                    )

                    t1 = wb_pool.tile([128, N], DT)
                    p2 = None if join == "hybrid" else wb_pool.tile([128, N], DT)
                    if join == "hybrid":
                        # t1 = (sqy-1)*e via 4x TS + 2x TT; the final image
                        # joins via STT+accum (in-op, no tail), earlier
                        # images via TT + ACT Copy accum (overlaps later
                        # images' DVE work).  p2 reuses dead buffers (sqy /
                        # qy) so two images fit in SBUF with bufs=2.
                        qy = wb_pool.tile([128, N], DT)
                        nc.vector.tensor_scalar(
                            qy[:], sqy[:], 1.0, None,
                            op0=mybir.AluOpType.subtract,
                        )
                        nc.vector.tensor_mul(t1[:], qy[:], e_flat)
                        if b < Bc - 1:
                            qx = wb_pool.tile([128, N], DT)
                            nc.vector.tensor_scalar(
                                qx[:], sqx[:], 1.0, None,
                                op0=mybir.AluOpType.subtract,
                            )
                            nc.vector.tensor_mul(sqy[:], qx[:], t1[:])
                            nc.scalar.activation(
                                qx[:], sqy[:],
                                mybir.ActivationFunctionType.Copy,
                                accum_out=acc_p2[:, 2 * b : 2 * b + 1],
                            )
                        else:
                            # final image: slice the 1x STT (and its own
                            # accum column) per live range -- the skipped
                            # dead-slot terms are exactly zero
                            sqx4 = sqx[:].rearrange("p (s z) -> p s z", s=WS)
                            t14 = t1[:].rearrange("p (s z) -> p s z", s=WS)
                            qy4 = qy[:].rearrange("p (s z) -> p s z", s=WS)
                            for ri, (s0, w) in enumerate(rngs):
                                nc.vector.scalar_tensor_tensor(
                                    qy4[:, s0 : s0 + w],
                                    sqx4[:, s0 : s0 + w], 1.0,
                                    t14[:, s0 : s0 + w],
                                    op0=mybir.AluOpType.subtract,
                                    op1=mybir.AluOpType.mult,
                                    accum_out=acc_p2[:, 2 * b + ri : 2 * b + ri + 1],
                                )
                    elif join == "stt":
                        nc.vector.scalar_tensor_tensor(
                            t1[:], sqy[:], 1.0, e_flat,
                            op0=mybir.AluOpType.subtract,
                            op1=mybir.AluOpType.mult,
                        )
                        nc.vector.scalar_tensor_tensor(
                            p2[:], sqx[:], 1.0, t1[:],
                            op0=mybir.AluOpType.subtract,
                            op1=mybir.AluOpType.mult,
                            accum_out=acc_p2[:, b : b + 1],
                        )
                    elif join == "ttr":
                        nc.vector.scalar_tensor_tensor(
                            t1[:], sqy[:], 1.0, e_flat,
                            op0=mybir.AluOpType.subtract,
                            op1=mybir.AluOpType.mult,
                            accum_out=acc_t1[:, b : b + 1],
                        )
                        dummy = wb_pool.tile([128, 1], DT)
                        nc.vector.tensor_tensor_reduce(
                            dummy[:].broadcast_to((128, N)),
                            sqx[:], t1[:],
                            scale=1.0, scalar=0.0,
                            op0=mybir.AluOpType.mult,
                            op1=mybir.AluOpType.add,
                            accum_out=acc_p2[:, b : b + 1],
                        )
                    elif join == "act":
                        qy = wb_pool.tile([128, N], DT)
                        qx = wb_pool.tile([128, N], DT)
                        nc.vector.tensor_scalar(
                            qy[:], sqy[:], 1.0, None,
                            op0=mybir.AluOpType.subtract,
                        )
                        nc.vector.tensor_mul(t1[:], qy[:], e_flat)
                        nc.vector.tensor_scalar(
                            qx[:], sqx[:], 1.0, None,
                            op0=mybir.AluOpType.subtract,
                        )
                        nc.vector.tensor_mul(p2[:], qx[:], t1[:])
                        nc.scalar.activation(
                            t1[:], p2[:],
                            mybir.ActivationFunctionType.Copy,
                            accum_out=acc_p2[:, b : b + 1],
                        )
                    else:
                        raise ValueError(join)

                # total; ttr accumulated sum(sqx*t1) so subtract sum(t1)
                red_p = c_pool.tile([128, 1], F32)
                nc.vector.reduce_sum(red_p[:], acc_p2[:], axis=mybir.AxisListType.X)
                if join == "ttr":
                    red_t = c_pool.tile([128, 1], F32)
                    nc.vector.reduce_sum(
                        red_t[:], acc_t1[:], axis=mybir.AxisListType.X
                    )
                    nc.vector.tensor_sub(red_p[:], red_p[:], red_t[:])
                out_ps = ps_pool.tile([1, 1], F32)
                nc.tensor.matmul(out_ps[:], red_p[:], ones_t[:], start=True, stop=True)
                out_sb = c_pool.tile([1, 1], F32)
                nc.vector.tensor_copy(out_sb[:], out_ps[:])
                nc.sync.dma_start(out_h.ap(), out_sb[:])

            if niter == 1:
                emit_iter()
            else:
                with tc.For_i(0, niter, 1):
                    emit_iter()

    nc.compile()
    return nc


def _pack_host(mask, edge, idx, dt_name="float16"):
    """Pack f32 [B,X,Y,{4,1}] inputs to the device layouts."""
    npdt = _np_dtype(dt_name)
    B, X, Y, _ = mask.shape
    KR, KC, live_s, live_j, WJ = _geom(idx)
    erow = WJ * YG
    m = mask[..., idx].reshape(B, NG, 8, Y)[:, :, KR, :]
    m = m.reshape(B, 7 * NG, YG, 8)[..., KC]
    mdev = np.zeros((B, 897, MROW), npdt)
    mdev[:, :896, :896] = m.reshape(B, 896, 896).astype(npdt)
    # edge: rows 7g+s aligned with mask slot s; dead slots stay zero.
    # col slot jj maps to orig class KC[jj] (WJ==7) or KC[live_j[jj]]
    # (WJ==6); dead-term columns stay zero.
    e = edge[..., 0]
    EC = list(range(7)) if WJ == 7 else live_j
    dead_cols = [jj for jj in range(WJ) if EC[jj] not in live_j]
    edev = np.zeros((B, NG, 7, erow), npdt)
    for s in live_s:
        c = KR[s]
        es = e.reshape(B, NG, 8, Y)[:, :, c, :].reshape(B, NG, YG, 8)
        es = es[..., [KC[jj] for jj in EC]].astype(npdt)
        if dead_cols:
            es[..., dead_cols] = 0
        edev[:, :, s, :] = es.reshape(B, NG, erow)
    return mdev, np.ascontiguousarray(edev.reshape(B, ENROW, erow))


def _run(mask, edge, loss_old, idx, trace=False, niter=1, join="hybrid",
         dt_name=None):
    if dt_name is None:
        dt_name = os.environ.get("KDT", "float16")
    B, X, Y, _ = mask.shape
    assert B % N_CORES == 0
    Bc = B // N_CORES

    nc = _build_program(Bc, idx, niter=niter, join=join, dt_name=dt_name)
    mdev, edev = _pack_host(mask, edge, idx, dt_name)
    in_maps = [
        {
            "mask": mdev[i * Bc : (i + 1) * Bc],
            "edge": edev[i * Bc : (i + 1) * Bc],
        }
        for i in range(N_CORES)
    ]
    res = run_bass_kernel_spmd(nc, in_maps, list(range(N_CORES)), trace=trace)
    total = float(sum(float(res.results[i]["out"][0, 0]) for i in range(N_CORES)))
    n_patch = ((X + 8) // 8) * ((Y + 8) // 8)
    out = np.float32(np.asarray(loss_old, dtype=np.float32) + total / (B * n_patch))
    return np.asarray(out, dtype=np.float32), res


def kernel(resized_image, mask_combined, edge_map, loss_old, mask_index):
    mask = np.ascontiguousarray(np.asarray(mask_combined, dtype=np.float32))
    edge = np.ascontiguousarray(np.asarray(edge_map, dtype=np.float32))
    idx = int(np.asarray(mask_index))
    out, _ = _run(mask, edge, loss_old, idx)
    return out
